# revision 1
# baseline (speedup 1.0000x reference)
"""Trainium2 Bass kernel for nn_AttentionBlock (GroupNorm -> qkv -> 4-head
attention over 1024 tokens -> proj -> residual), B=16, C=256, H=W=32.

Sharding: data-parallel over batch across 8 NeuronCores (2 batches/core).
Inside each core: matmuls run in float32r (full PE rate); scores are computed
transposed and row-packed per head pair; exp on ScalarE straight from PSUM;
the transposed-v operand carries a ones-column so attn@v also produces the
softmax denominator; GroupNorm rstd is folded into the qkv weights per batch
and all static biases are folded on the host. Head-pair software pipeline
interleaves each slot's attn@v into the next slot's score stream.

Self-contained: callable as kernel(**inputs) with the full unsharded inputs.
"""
"""Builder for the AttentionBlock trn2 kernel (per-core program, SPMD x8).

Per-core work: 2 batches of GroupNorm -> qkv -> 4-head attention -> proj -> residual.

Pipeline: heads are processed in pairs (4 pair-slots per iteration). A slot's
row-packed score matmuls + exps run ACT-paced; the previous slot's attn@v
blocks and normalization are interleaved between them so PE/DVE stay busy.

Layouts:
  x_sb[t]   : [128, 1024] f32r  (channels on partitions, tokens free)
  qk_sb[ot] : [128, 1024] f32r  (ot 0..1 = q channels, 2..3 = k channels)
  vT_sb[nt] : [128, 4, 65] bf16 (token-tile on partitions; 64 v-ch + ones col)
  PT[h][jt] : [128, 1024] bf16  exp(scores^T)
  o_ps      : [65, 512] f32 psum (64 head channels + denominator row)
  O_all[t]  : [128, 1024] f32r  normalized attention output
"""
import contextlib
import numpy as np
import concourse.bacc as bacc
import concourse.tile as tile
from concourse import mybir

F32 = mybir.dt.float32
F32R = mybir.dt.float32r
BF16 = mybir.dt.bfloat16
AF = mybir.ActivationFunctionType
ALU = mybir.AluOpType

B_PER_CORE = 2
EPS = 1e-5


def build(nbatch=B_PER_CORE, loop_reps=1):
    nc = bacc.Bacc("TRN2", target_bir_lowering=False, debug=False)

    x_d = nc.dram_tensor("x", [nbatch, 256, 32, 32], F32R, kind="ExternalInput")
    wqkvT_d = nc.dram_tensor("wqkvT", [256, 768], F32R, kind="ExternalInput")
    projT_d = nc.dram_tensor("projT", [256, 256], F32R, kind="ExternalInput")
    cbeta_d = nc.dram_tensor("cbeta", [128, 6], F32, kind="ExternalInput")
    cproj_d = nc.dram_tensor("cproj", [128, 2], F32, kind="ExternalInput")
    sel_d = nc.dram_tensor("sel", [2, 128, 32], F32R, kind="ExternalInput")
    selT_d = nc.dram_tensor("selT", [2, 32, 128], F32R, kind="ExternalInput")
    ones_d = nc.dram_tensor("ones", [128, 128], F32R, kind="ExternalInput")
    ones16_d = nc.dram_tensor("ones16", [128, 4], BF16, kind="ExternalInput")
    y_d = nc.dram_tensor("y", [nbatch, 256, 32, 32], F32, kind="ExternalOutput")

    x_ap = x_d.ap().rearrange("b c h w -> b c (h w)")
    y_ap = y_d.ap().rearrange("b c h w -> b c (h w)")

    with tile.TileContext(nc) as tc:
        with tc.tile_pool(name="const", bufs=1) as constp, \
             tc.tile_pool(name="xp", bufs=2) as xp, \
             tc.tile_pool(name="wsp", bufs=2) as wsp, \
             tc.tile_pool(name="qkp", bufs=2) as qkp, \
             tc.tile_pool(name="vtp", bufs=2) as vtp, \
             tc.tile_pool(name="ptp", bufs=32) as ptp, \
             tc.tile_pool(name="oap", bufs=2) as oap, \
             tc.tile_pool(name="smallp", bufs=4) as smallp, \
             tc.tile_pool(name="yp", bufs=2) as yp, \
             tc.tile_pool(name="pss", bufs=2, space="PSUM") as pss, \
             tc.tile_pool(name="pso", bufs=2, space="PSUM") as pso, \
             tc.tile_pool(name="psw", bufs=2, space="PSUM") as psw:

            # ---- constants (loaded once) ----
            wqkvT_t = [constp.tile([128, 768], F32R, name=f"wqkvT{t}") for t in range(2)]
            projT_t = [constp.tile([128, 256], F32R, name=f"projT{t}") for t in range(2)]
            cbeta_t = constp.tile([128, 6], F32)
            cproj_t = constp.tile([128, 2], F32)
            sel_t = [constp.tile([128, 32], F32R, name=f"sel{t}") for t in range(2)]
            selT_t = [constp.tile([32, 128], F32R, name=f"selT{t}") for t in range(2)]
            ones_t = constp.tile([128, 128], F32R)
            for t in range(2):
                nc.sync.dma_start(out=wqkvT_t[t][:, :], in_=wqkvT_d.ap()[128*t:128*(t+1), :])
                nc.sync.dma_start(out=projT_t[t][:, :], in_=projT_d.ap()[128*t:128*(t+1), :])
                nc.sync.dma_start(out=sel_t[t][:, :], in_=sel_d.ap()[t, :, :])
                nc.sync.dma_start(out=selT_t[t][:, :], in_=selT_d.ap()[t, :, :])
            nc.sync.dma_start(out=cbeta_t[:, :], in_=cbeta_d.ap()[:, :])
            nc.sync.dma_start(out=cproj_t[:, :], in_=cproj_d.ap()[:, :])
            nc.sync.dma_start(out=ones_t[:, :], in_=ones_d.ap()[:, :])

            def emit_prelude(b, sfx):
                S = {"b": b, "sfx": sfx}
                x_t = [xp.tile([128, 1024], F32R, name=f"x{sfx}t{t}", tag=f"x{t}")
                       for t in range(2)]
                for t in range(2):
                    nc.sync.dma_start(out=x_t[t][:, :], in_=x_ap[b, 128*t:128*(t+1), :])
                S["x"] = x_t

                # GroupNorm stats
                m2mv = []
                for t in range(2):
                    stats = smallp.tile([128, 2, 6], F32, name=f"st{sfx}t{t}", tag="stats")
                    xf = x_t[t][:, :].bitcast(F32).rearrange("p (s n) -> p s n", s=2)
                    nc.vector.bn_stats(out=stats[:, 0, :], in_=xf[:, 0, :])
                    nc.vector.bn_stats(out=stats[:, 1, :], in_=xf[:, 1, :])
                    mv = smallp.tile([128, 2], F32, name=f"mv{sfx}t{t}", tag="mv")
                    nc.vector.bn_aggr(out=mv[:, :], in_=stats[:, :, :])
                    mm = smallp.tile([128, 2], F32R, name=f"mm{sfx}t{t}", tag="mm")
                    nc.vector.tensor_copy(out=mm[:, 0:1], in_=mv[:, 0:1])
                    nc.vector.tensor_scalar(out=mm[:, 1:2], in0=mv[:, 0:1],
                                            scalar1=mv[:, 0:1], scalar2=mv[:, 1:2],
                                            op0=ALU.mult, op1=ALU.add)
                    m2mv.append(mm)
                gstat_ps = psw.tile([32, 2], F32, name=f"gst{sfx}", tag="w")
                for t in range(2):
                    nc.tensor.matmul(gstat_ps[:, :], sel_t[t][:, :].bitcast(F32),
                                     m2mv[t][:, :].bitcast(F32),
                                     start=(t == 0), stop=(t == 1))

                # group mean / rstd (Newton rsqrt; var ~ 1)
                gmu = smallp.tile([32, 2], F32R, name=f"gmu{sfx}", tag="gmu")
                nc.vector.tensor_scalar(out=gmu[:, 0:1], in0=gstat_ps[:, 0:1],
                                        scalar1=0.125, scalar2=None, op0=ALU.mult)
                ta = smallp.tile([32, 4], F32, name=f"ta{sfx}", tag="ta")
                nc.vector.tensor_scalar(out=ta[:, 0:1], in0=gstat_ps[:, 1:2],
                                        scalar1=0.125, scalar2=EPS,
                                        op0=ALU.mult, op1=ALU.add)
                gmuf = gmu[:, 0:1].bitcast(F32)
                nc.vector.tensor_scalar(out=ta[:, 1:2], in0=gmuf, scalar1=gmuf,
                                        scalar2=None, op0=ALU.mult)
                nc.vector.tensor_tensor(out=ta[:, 2:3], in0=ta[:, 0:1], in1=ta[:, 1:2],
                                        op=ALU.subtract)
                nc.vector.tensor_scalar(out=ta[:, 3:4], in0=ta[:, 2:3],
                                        scalar1=-0.5, scalar2=1.5,
                                        op0=ALU.mult, op1=ALU.add)
                for it in range(3):
                    tb = smallp.tile([32, 3], F32, name=f"tb{sfx}i{it}", tag="tb")
                    nc.vector.tensor_tensor(out=tb[:, 0:1], in0=ta[:, 3:4],
                                            in1=ta[:, 3:4], op=ALU.mult)
                    nc.vector.tensor_tensor(out=tb[:, 1:2], in0=tb[:, 0:1],
                                            in1=ta[:, 2:3], op=ALU.mult)
                    nc.vector.tensor_scalar(out=tb[:, 2:3], in0=tb[:, 1:2],
                                            scalar1=-0.5, scalar2=1.5,
                                            op0=ALU.mult, op1=ALU.add)
                    if it < 2:
                        ta2 = smallp.tile([32, 4], F32, name=f"ta{sfx}i{it}", tag="ta")
                        nc.vector.tensor_copy(out=ta2[:, 2:3], in_=ta[:, 2:3])
                        nc.vector.tensor_tensor(out=ta2[:, 3:4], in0=ta[:, 3:4],
                                                in1=tb[:, 2:3], op=ALU.mult)
                        ta = ta2
                    else:
                        nc.vector.tensor_tensor(out=gmu[:, 1:2], in0=ta[:, 3:4],
                                                in1=tb[:, 2:3], op=ALU.mult)

                # broadcast (mu, rstd) to channels; scale W; biases
                chs = []
                for t in range(2):
                    ch_ps = psw.tile([128, 2], F32, name=f"chp{sfx}t{t}", tag="w")
                    nc.tensor.matmul(ch_ps[:, :], selT_t[t][:, :].bitcast(F32),
                                     gmu[:, :].bitcast(F32), start=True, stop=True)
                    ch = smallp.tile([128, 2], F32R, name=f"chs{sfx}t{t}", tag="chs")
                    nc.vector.tensor_copy(out=ch[:, :], in_=ch_ps[:, :])
                    chs.append(ch)
                wq_s = []
                for t in range(2):
                    ws = wsp.tile([128, 768], F32R, name=f"ws{sfx}t{t}", tag=f"ws{t}")
                    nc.vector.tensor_scalar(out=ws[:, :],
                                            in0=wqkvT_t[t][:, :].bitcast(F32),
                                            scalar1=chs[t][:, 1:2].bitcast(F32),
                                            scalar2=None, op0=ALU.mult)
                    wq_s.append(ws)
                bias_ps = psw.tile([128, 6], F32, name=f"bps{sfx}", tag="w")
                for h in range(6):
                    for t in range(2):
                        nc.tensor.matmul(bias_ps[:, h:h+1],
                                         wq_s[t][:, 128*h:128*(h+1)].bitcast(F32),
                                         chs[t][:, 0:1].bitcast(F32),
                                         start=(t == 0), stop=(t == 1))
                bias_sb = smallp.tile([128, 6], F32R, name=f"bsb{sfx}", tag="bsb")
                nc.vector.tensor_tensor(out=bias_sb[:, :], in0=cbeta_t[:, :],
                                        in1=bias_ps[:, :], op=ALU.subtract)
                # proj-side correction for the v bias
                pb_ps = psw.tile([128, 2], F32, name=f"pbps{sfx}", tag="w")
                for ot in range(2):
                    for t in range(2):
                        nc.tensor.matmul(pb_ps[:, ot:ot+1],
                                         projT_t[t][:, 128*ot:128*(ot+1)].bitcast(F32),
                                         bias_sb[:, 4+t:5+t].bitcast(F32),
                                         start=(t == 0), stop=(t == 1))
                pbias_sb = smallp.tile([128, 2], F32, name=f"pbias{sfx}", tag="pbias")
                nc.vector.tensor_tensor(out=pbias_sb[:, :], in0=cproj_t[:, :],
                                        in1=pb_ps[:, :], op=ALU.add)
                S["pbias"] = pbias_sb

                # qkv: q, k (channel-major)
                qk_sb = []
                for ot in range(4):
                    qk = qkp.tile([128, 1024], F32R, name=f"qk{sfx}o{ot}", tag=f"qk{ot}")
                    qk_sb.append(qk)
                    for ih in range(2):
                        q_ps = pso.tile([128, 512], F32, name=f"qps{sfx}o{ot}i{ih}",
                                        tag="o")
                        for t in range(2):
                            nc.tensor.matmul(q_ps[:, :],
                                             wq_s[t][:, 128*ot:128*(ot+1)],
                                             x_t[t][:, 512*ih:512*(ih+1)],
                                             start=(t == 0), stop=(t == 1))
                        nc.vector.tensor_scalar(out=qk[:, 512*ih:512*(ih+1)],
                                                in0=q_ps[:, :],
                                                scalar1=bias_sb[:, ot:ot+1].bitcast(F32),
                                                scalar2=None, op0=ALU.add)
                S["qk"] = qk_sb

                # vT (token-major, bf16) + ones column
                vT_sb = []
                for nt in range(8):
                    vt = vtp.tile([128, 4, 65], BF16, name=f"vt{sfx}n{nt}", tag=f"vt{nt}")
                    vT_sb.append(vt)
                    nc.sync.dma_start(out=vt[:, :, 64:65], in_=ones16_d.ap()[:, 0:4])
                    vt_ps = pso.tile([128, 256], F32, name=f"vps{sfx}n{nt}", tag="o")
                    for t in range(2):
                        nc.tensor.matmul(vt_ps[:, :],
                                         x_t[t][:, 128*nt:128*(nt+1)],
                                         wq_s[t][:, 512:768],
                                         start=(t == 0), stop=(t == 1))
                    nc.vector.tensor_copy(
                        out=vt[:, :, 0:64],
                        in_=vt_ps[:, :].rearrange("p (h d) -> p h d", h=4))
                S["vt"] = vT_sb
                S["pts"] = {h: [] for h in range(4)}
                S["oall"] = [oap.tile([128, 1024], F32R, name=f"oall{sfx}t{t}",
                                      tag=f"oall{t}") for t in range(2)]
                return S

            def emit_scores(S, pair, jt):
                sfx = S["sfx"]
                qk_sb = S["qk"]
                s_ps = {}
                for hh in range(2):
                    h = 2 * pair + hh
                    s_ps[hh] = pss.tile([128, 1024], F32, name=f"sps{sfx}h{h}j{jt}",
                                        tag="s")
                for ih in range(2):
                    for hh in range(2):
                        nc.tensor.matmul(
                            s_ps[hh][:, 512*ih:512*(ih+1)],
                            qk_sb[2 + pair][64*hh:64*hh+64, 128*jt:128*(jt+1)],
                            qk_sb[pair][64*hh:64*hh+64, 512*ih:512*(ih+1)],
                            start=True, stop=True,
                            tile_position=(64*hh, 0))
                for hh in range(2):
                    h = 2 * pair + hh
                    pt = ptp.tile([128, 1024], BF16, name=f"pt{sfx}h{h}j{jt}", tag="pt")
                    nc.scalar.activation(out=pt[:, :], in_=s_ps[hh][:, :], func=AF.Exp,
                                         scale=0.125)
                    S["pts"][h].append(pt)

            def emit_attnv_norm(S, h, ih):
                sfx = S["sfx"]
                pts = S["pts"][h]
                o_ps = pso.tile([65, 512], F32, name=f"ops{sfx}h{h}i{ih}", tag="o")
                for jt in range(8):
                    nc.tensor.matmul(o_ps[:, :],
                                     S["vt"][jt][:, h, :],
                                     pts[jt][:, 512*ih:512*(ih+1)],
                                     start=(jt == 0), stop=(jt == 7))
                od = smallp.tile([65, 512], F32R, name=f"od{sfx}h{h}i{ih}", tag="od")
                nc.vector.tensor_copy(out=od[:, :], in_=o_ps[:, :])
                r_ps = psw.tile([64, 512], F32, name=f"rps{sfx}h{h}i{ih}", tag="w")
                nc.tensor.matmul(r_ps[:, :], ones_t[64:65, 0:64], od[64:65, :],
                                 start=True, stop=True)
                rr = smallp.tile([64, 512], F32, name=f"rr{sfx}h{h}i{ih}", tag="rr")
                nc.vector.reciprocal_approx_fast(out=rr[:, :], in_=r_ps[:, :])
                nc.vector.tensor_tensor(
                    out=S["oall"][h // 2][64*(h % 2):64*(h % 2)+64,
                                          512*ih:512*(ih+1)],
                    in0=od[0:64, :].bitcast(F32), in1=rr[:, :], op=ALU.mult)

            def emit_proj(S):
                sfx = S["sfx"]
                b = S["b"]
                y_sb = [yp.tile([128, 1024], F32, name=f"y{sfx}t{t}", tag=f"y{t}")
                        for t in range(2)]
                for ot in range(2):
                    for ih in range(2):
                        p_ps = pso.tile([128, 512], F32, name=f"pps{sfx}o{ot}i{ih}",
                                        tag="o")
                        for t in range(2):
                            nc.tensor.matmul(p_ps[:, :],
                                             projT_t[t][:, 128*ot:128*(ot+1)],
                                             S["oall"][t][:, 512*ih:512*(ih+1)],
                                             start=(t == 0), stop=(t == 1))
                        nc.vector.affine_then_add(
                            out=y_sb[ot][:, 512*ih:512*(ih+1)], in0=p_ps[:, :],
                            in1=S["x"][ot][:, 512*ih:512*(ih+1)].bitcast(F32),
                            scale=1.0, bias=S["pbias"][:, ot:ot+1])
                for ot in range(2):
                    nc.sync.dma_start(out=y_ap[b, 128*ot:128*(ot+1), :],
                                      in_=y_sb[ot][:, :])

            loop_ctx = (tc.For_i(0, loop_reps, 1,
                                 hint_engines=(mybir.EngineType.PE,))
                        if loop_reps > 1 else contextlib.nullcontext())
            with loop_ctx:
                slots = [(b, p) for b in range(nbatch) for p in range(2)]
                S_by_b = {}
                prev = None
                for (b, pair) in slots:
                    if pair == 0:
                        S_by_b[b] = emit_prelude(b, f"b{b}")
                    for jt in range(8):
                        emit_scores(S_by_b[b], pair, jt)
                        if prev is not None and jt % 2 == 1:
                            pb, pp = prev
                            blk = jt // 2
                            emit_attnv_norm(S_by_b[pb], 2 * pp + blk // 2, blk % 2)
                            if blk == 3 and pp == 1:
                                emit_proj(S_by_b[pb])
                    prev = (b, pair)
                pb, pp = prev
                for blk in range(4):
                    emit_attnv_norm(S_by_b[pb], 2 * pp + blk // 2, blk % 2)
                emit_proj(S_by_b[pb])

    nc.compile()
    return nc


def host_constants(gn_w, gn_b, qkv_w, qkv_b, proj_w, proj_b):
    """Fold static parameters into the tensors the kernel expects."""
    import ml_dtypes
    wqkv = qkv_w * gn_w[None, :]             # [768, 256]
    cbeta = qkv_w @ gn_b + qkv_b             # [768]
    sel = np.zeros((2, 128, 32), np.float32)
    selT = np.zeros((2, 32, 128), np.float32)
    for t in range(2):
        for c in range(128):
            g = (128 * t + c) // 8
            sel[t, c, g] = 1.0
            selT[t, g, c] = 1.0
    return {
        "wqkvT": np.ascontiguousarray(wqkv.T).astype(np.float32),   # [256, 768]
        "projT": np.ascontiguousarray(proj_w.T).astype(np.float32), # [256, 256]
        "cbeta": np.ascontiguousarray(cbeta.reshape(6, 128).T).astype(np.float32),
        "cproj": np.ascontiguousarray(proj_b.reshape(2, 128).T).astype(np.float32),
        "sel": sel, "selT": selT,
        "ones": np.ones((128, 128), np.float32),
        "ones16": np.ones((128, 4), ml_dtypes.bfloat16),
    }


_CACHE = {}


def kernel(x, gn_w, gn_b, qkv_w, qkv_b, proj_w, proj_b):
    from concourse.bass_utils import run_bass_kernel_spmd

    x = np.asarray(x, dtype=np.float32)
    consts = host_constants(np.asarray(gn_w, np.float32), np.asarray(gn_b, np.float32),
                            np.asarray(qkv_w, np.float32), np.asarray(qkv_b, np.float32),
                            np.asarray(proj_w, np.float32), np.asarray(proj_b, np.float32))
    if "nc" not in _CACHE:
        _CACHE["nc"] = build()
    nc = _CACHE["nc"]
    n_cores = 8
    bpc = x.shape[0] // n_cores
    in_maps = [{"x": np.ascontiguousarray(x[bpc*i:bpc*(i+1)]), **consts}
               for i in range(n_cores)]
    res = run_bass_kernel_spmd(nc, in_maps, core_ids=list(range(n_cores)))
    return np.concatenate([res.results[i]["y"] for i in range(n_cores)],
                          axis=0).astype(np.float32)

